# revision 22
# baseline (speedup 1.0000x reference)
"""Multi-head causal attention (RoPE + QK-RMSNorm) on 8 TRN2 NeuronCores.

Sharding: data parallel on batch (2) x tensor parallel on heads (4 groups of
2 heads).  core = 4*b + g computes, for batch b, heads [2g, 2g+1]:
  q/k/v projections (E-sliced), qk-rmsnorm + rope, causal attention, and the
  Wo partial product over its E slice.  Host sums the 4 partials per batch.

Everything on device runs in the "transposed" orientation:
  qT/kT [e, s], v [s, e], scoresT [sk, sq], out_T [d_out, s]
so no on-chip data transposes are needed; softmax denominators and rms sums
are computed with ones-matmuls on the TensorEngine in *column* layout
[128 positions, few cols].  1/x uses the DVE divide unit; 1/sqrt(x) uses a
sqrt bit-hack seed + 2 Babylonian iterations + reciprocal, all on DVE, so
the ScalarEngine only ever needs the exp table set (no table thrashing).
Per-position column scales are turned into [1, n] rows via a small PE
transpose and broadcast across partitions with gpsimd.partition_broadcast.
"""

import math

import numpy as np
import ml_dtypes

import concourse.bass as bass
import concourse.tile as tile
from concourse import bacc, mybir
from concourse.bass_utils import run_bass_kernel_spmd

# Problem shapes (hardcoded per instructions).
B = 2
S = 2048
D = 2048
H = 8
HD = 256
HALF = 128
EL = 512          # E columns per core (2 heads)
CH = 512          # sq chunk size
NCH = S // CH     # 4
DT = D // 128     # 16 k-tiles over D
ET = EL // 128    # 4 e-tiles
ST = S // 128     # 16 s-tiles
EPS = 1e-6
N_CORES = 8

BF16 = mybir.dt.bfloat16
F32 = mybir.dt.float32
I32 = mybir.dt.int32
NBF = ml_dtypes.bfloat16

SQRT_MAGIC = 0x1FBD1DF5     # sqrt(x) seed: (bits(x) >> 1) + MAGIC

_CACHE: dict = {}


def _build(reps: int = 1):
    nc = bacc.Bacc("TRN2", target_bir_lowering=False, debug=False,
                   num_devices=N_CORES)

    xT_d = nc.dram_tensor("xT", [D, S], BF16, kind="ExternalInput").ap()
    wq_d = nc.dram_tensor("wqT", [D, EL], BF16, kind="ExternalInput").ap()
    wk_d = nc.dram_tensor("wkT", [D, EL], BF16, kind="ExternalInput").ap()
    wv_d = nc.dram_tensor("wvT", [D, EL], BF16, kind="ExternalInput").ap()
    wo_d = nc.dram_tensor("woT", [EL, D], BF16, kind="ExternalInput").ap()
    rtq_d = nc.dram_tensor("rtq", [4, HALF, S], BF16, kind="ExternalInput").ap()
    rtk_d = nc.dram_tensor("rtk", [4, HALF, S], BF16, kind="ExternalInput").ap()
    msk_d = nc.dram_tensor("masks", [4, HALF, CH], BF16, kind="ExternalInput").ap()
    out_d = nc.dram_tensor("outT", [D, S], F32, kind="ExternalOutput").ap()

    with tile.TileContext(nc) as tc:
        for _ in range(reps):
            _emit(tc, nc, xT_d, wq_d, wk_d, wv_d, wo_d, rtq_d, rtk_d, msk_d,
                  out_d)
    nc.compile()
    return nc


def _emit(tc, nc, xT_d, wq_d, wk_d, wv_d, wo_d, rtq_d, rtk_d, msk_d, out_d):
    from contextlib import ExitStack
    ctx = ExitStack()
    with ctx:
        persist = ctx.enter_context(tc.tile_pool(name="persist", bufs=1))
        xs_p = ctx.enter_context(tc.tile_pool(name="xs", bufs=18))
        rt_p = ctx.enter_context(tc.tile_pool(name="rt", bufs=1))
        sq_p = ctx.enter_context(tc.tile_pool(name="sq", bufs=5))
        qr_p = ctx.enter_context(tc.tile_pool(name="qr", bufs=6))
        qs_p = ctx.enter_context(tc.tile_pool(name="qs", bufs=4))
        rtmp_p = ctx.enter_context(tc.tile_pool(name="rtmp", bufs=6))
        rb_p = ctx.enter_context(tc.tile_pool(name="rb", bufs=3))
        rd_p = ctx.enter_context(tc.tile_pool(name="rd", bufs=2))
        e_p = ctx.enter_context(tc.tile_pool(name="ep", bufs=8))
        o_p = ctx.enter_context(tc.tile_pool(name="op", bufs=3))
        nr_p = ctx.enter_context(tc.tile_pool(name="nrp", bufs=3))
        at_p = ctx.enter_context(tc.tile_pool(name="atp", bufs=2))

        ps_big = ctx.enter_context(tc.tile_pool(name="psb", bufs=6, space="PSUM"))
        ps_col = ctx.enter_context(tc.tile_pool(name="psc", bufs=2, space="PSUM"))

        # ---- persistent tiles ----
        wq_sb = persist.tile([128, DT, EL], BF16, tag="wq")
        wk_sb = persist.tile([128, DT, EL], BF16, tag="wk")
        wv_sb = persist.tile([128, DT, EL], BF16, tag="wv")
        wo_sb = persist.tile([128, ET, D], BF16, tag="wo")
        qT_sb = persist.tile([128, ET, S], BF16, tag="qT")
        kT_sb = persist.tile([128, ET, S], BF16, tag="kT")
        v_sb = persist.tile([128, ST, EL], BF16, tag="v")
        msk_sb = persist.tile([128, 4, CH], BF16, tag="msk")
        ones_sb = persist.tile([128, 1], BF16, tag="ones")
        rk_sb = persist.tile([128, 2, ST], F32, tag="rk")   # f_k per (head, sk)
        nc.vector.memset(ones_sb, 1.0)

        nc.sync.dma_start(out=wq_sb, in_=wq_d.rearrange("(t p) e -> p t e", p=128))

        def load_wk():
            nc.sync.dma_start(out=wk_sb, in_=wk_d.rearrange("(t p) e -> p t e", p=128))

        def load_wv():
            nc.sync.dma_start(out=wv_sb, in_=wv_d.rearrange("(t p) e -> p t e", p=128))

        def load_rest():
            nc.sync.dma_start(out=wo_sb, in_=wo_d.rearrange("(t p) d -> p t d", p=128))
            nc.sync.dma_start(out=msk_sb, in_=msk_d.rearrange("t p s -> p t s"))

        LN_EPS = float(HD * EPS)

        def nr_rsqrt(dst, src_ps, n, scale16):
            """dst[128, n] = (src_ps + 256*eps)^(-1/2) (*16), DVE only."""
            x = nr_p.tile([128, 8], F32, tag="nrx", name=f"nrx{nr_rsqrt.i}")[:, :n]
            nc.vector.tensor_scalar(out=x, in0=src_ps, scalar1=LN_EPS,
                                    scalar2=None, op0=mybir.AluOpType.add)
            s = nr_p.tile([128, 8], F32, tag="nry", name=f"nry{nr_rsqrt.i}")[:, :n]
            nc.vector.tensor_scalar(
                out=s.bitcast(I32), in0=x.bitcast(I32), scalar1=1,
                scalar2=None, op0=mybir.AluOpType.arith_shift_right)
            nc.vector.tensor_scalar(
                out=s.bitcast(I32), in0=s.bitcast(I32), scalar1=SQRT_MAGIC,
                scalar2=None, op0=mybir.AluOpType.add)
            for it in range(2):
                r = nr_p.tile([128, 8], F32, tag="nrt",
                              name=f"nrt{nr_rsqrt.i}_{it}")[:, :n]
                nc.vector.reciprocal(out=r, in_=s)
                nc.vector.tensor_mul(out=r, in0=r, in1=x)     # x / s
                nc.vector.tensor_add(out=s, in0=s, in1=r)
                nc.vector.tensor_scalar(out=s, in0=s, scalar1=0.5,
                                        scalar2=None, op0=mybir.AluOpType.mult)
            nr_rsqrt.i += 1
            nc.vector.reciprocal(out=dst, in_=s)
            if scale16:
                nc.vector.tensor_scalar(out=dst, in0=dst, scalar1=16.0,
                                        scalar2=None, op0=mybir.AluOpType.mult)

        nr_rsqrt.i = 0

        def col_to_row(src, n):
            """[128, n] f32 cols -> [1, n*128] row on partition 0 (small DMA).

            row[0, 128*j + p] = src[p, j]"""
            row = nr_p.tile([1, 1024], F32, tag="row",
                            name=f"row{col_to_row.i}")
            col_to_row.i += 1
            for j in range(n):
                nc.gpsimd.dma_start(
                    out=row[:, j * 128:(j + 1) * 128], in_=src[:, j:j + 1])
            return row
        col_to_row.i = 0

        def _rope(dst_sb, et0, cs, x1, x2, tab):
            """dst[:, et0, cs]   = x1*tab[0] - x2*tab[3]   (cw1, sw2)
               dst[:, et0+1, cs] = x2*tab[2] + x1*tab[1]   (cw2, sw1)"""
            t1 = rtmp_p.tile([128, CH], BF16, tag="rtmp")
            t2 = rtmp_p.tile([128, CH], BF16, tag="rtmp")
            nc.vector.tensor_mul(out=t1, in0=x1, in1=tab[:, 0, :])
            nc.vector.tensor_mul(out=t2, in0=x2, in1=tab[:, 3, :])
            nc.vector.tensor_sub(out=dst_sb[:, et0, cs], in0=t1, in1=t2)
            t3 = rtmp_p.tile([128, CH], BF16, tag="rtmp")
            t4 = rtmp_p.tile([128, CH], BF16, tag="rtmp")
            nc.vector.tensor_mul(out=t3, in0=x2, in1=tab[:, 2, :])
            nc.vector.tensor_mul(out=t4, in0=x1, in1=tab[:, 1, :])
            nc.vector.tensor_add(out=dst_sb[:, et0 + 1, cs], in0=t3, in1=t4)

        def phase_a(c, first=False):
            """QKV projections + rmsnorm + rope for chunk c."""
            cs = slice(c * CH, (c + 1) * CH)
            xs = []
            for dt in range(DT):
                t = xs_p.tile([128, CH], BF16, tag="xs")
                nc.sync.dma_start(
                    out=t, in_=xT_d[dt * 128:(dt + 1) * 128, cs])
                xs.append(t)
            rtq_t = rt_p.tile([128, 4, CH], BF16, tag="rtq")
            nc.sync.dma_start(out=rtq_t,
                              in_=rtq_d[:, :, cs].rearrange("t p s -> p t s"))
            rtk_t = rt_p.tile([128, 4, CH], BF16, tag="rtk")
            nc.sync.dma_start(out=rtk_t,
                              in_=rtk_d[:, :, cs].rearrange("t p s -> p t s"))
            if first:
                load_wk()

            # ---------- q ----------
            rq_ps = ps_col.tile([128, 8], F32, tag="col")
            qr = []
            sqs = []
            for et in range(ET):
                q_ps = ps_big.tile([128, CH], F32, tag="big")
                for dt in range(DT):
                    nc.tensor.matmul(
                        q_ps, wq_sb[:, dt, et * 128:(et + 1) * 128], xs[dt],
                        start=(dt == 0), stop=(dt == DT - 1))
                t = qr_p.tile([128, CH], BF16, tag="qr")
                nc.scalar.copy(out=t, in_=q_ps)
                qr.append(t)
                sqt = sq_p.tile([128, CH], BF16, tag="sq")
                nc.vector.tensor_mul(out=sqt, in0=t, in1=t)
                sqs.append(sqt)
            for et in range(ET):
                hh = et // 2
                for j in range(4):
                    nc.tensor.matmul(
                        rq_ps[:, 4 * hh + j: 4 * hh + j + 1],
                        sqs[et][:, j * 128:(j + 1) * 128], ones_sb,
                        start=(et == 0 and j == 0),
                        stop=(et == ET - 1 and j == 3))
            rq_sb = nr_p.tile([128, 8], F32, tag="rq")
            nr_rsqrt(rq_sb, rq_ps, 8, scale16=False)
            t_row = col_to_row(rq_sb, 8)
            rb = []
            for hh in range(2):
                rbt = rb_p.tile([128, CH], F32, tag="rb")
                for j in range(4):
                    t = 4 * hh + j
                    nc.gpsimd.partition_broadcast(
                        rbt[:, j * 128:(j + 1) * 128],
                        t_row[0:1, t * 128:(t + 1) * 128])
                rb.append(rbt)
            if first:
                load_wv()
            for hh in range(2):
                q1 = qs_p.tile([128, CH], BF16, tag="qs")
                nc.vector.tensor_mul(out=q1, in0=qr[2 * hh], in1=rb[hh])
                q2 = qs_p.tile([128, CH], BF16, tag="qs")
                nc.vector.tensor_mul(out=q2, in0=qr[2 * hh + 1], in1=rb[hh])
                _rope(qT_sb, 2 * hh, cs, q1, q2, rtq_t)

            # ---------- k ----------
            rk_ps = ps_col.tile([128, 8], F32, tag="col")
            kr = []
            ksqs = []
            for et in range(ET):
                k_ps = ps_big.tile([128, CH], F32, tag="big")
                for dt in range(DT):
                    nc.tensor.matmul(
                        k_ps, wk_sb[:, dt, et * 128:(et + 1) * 128], xs[dt],
                        start=(dt == 0), stop=(dt == DT - 1))
                t = qr_p.tile([128, CH], BF16, tag="qr")
                nc.scalar.copy(out=t, in_=k_ps)
                kr.append(t)
                sqt = sq_p.tile([128, CH], BF16, tag="sq")
                nc.vector.tensor_mul(out=sqt, in0=t, in1=t)
                ksqs.append(sqt)
            for et in range(ET):
                hh = et // 2
                for j in range(4):
                    nc.tensor.matmul(
                        rk_ps[:, 4 * hh + j: 4 * hh + j + 1],
                        ksqs[et][:, j * 128:(j + 1) * 128], ones_sb,
                        start=(et == 0 and j == 0),
                        stop=(et == ET - 1 and j == 3))
            nr_rsqrt(rk_sb[:, 0, 4 * c:4 * c + 4], rk_ps[:, 0:4], 4,
                     scale16=True)
            nr_rsqrt(rk_sb[:, 1, 4 * c:4 * c + 4], rk_ps[:, 4:8], 4,
                     scale16=True)
            for hh in range(2):
                _rope(kT_sb, 2 * hh, cs, kr[2 * hh], kr[2 * hh + 1], rtk_t)

            # ---------- v ----------
            for st in range(4):
                v_ps = ps_big.tile([128, EL], F32, tag="big")
                for dt in range(DT):
                    nc.tensor.matmul(
                        v_ps, xs[dt][:, st * 128:(st + 1) * 128],
                        wv_sb[:, dt, :],
                        start=(dt == 0), stop=(dt == DT - 1))
                nc.scalar.copy(out=v_sb[:, 4 * c + st, :], in_=v_ps)
            if first:
                load_rest()

        def phase_b_pair(c, aT_t, filler=None):
            """Attention for chunk c, both local heads, skt-interleaved.

            filler: optional iterator yielding callables that emit a burst of
            independent PE work (used to keep the PE fed during the serial
            score->exp->mask->av chains)."""
            cs = slice(c * CH, (c + 1) * CH)
            n_sk = 4 * c + 4
            av_ps = {(hh, i): ps_big.tile([128, CH], F32, tag="big",
                                          name=f"av{c}_{hh}_{i}")
                     for hh in range(2) for i in range(2)}
            den_ps = {hh: ps_col.tile([128, 4], F32, tag="col",
                                      name=f"den{c}_{hh}")
                      for hh in range(2)}
            for skt in range(n_sk):
                first, last = (skt == 0), (skt == n_sk - 1)
                e_ts = {}
                for hh in range(2):
                    sc_ps = ps_big.tile([128, CH], F32, tag="big",
                                        name=f"sc{c}_{hh}_{skt}")
                    for half in range(2):
                        et = 2 * hh + half
                        nc.tensor.matmul(
                            sc_ps, kT_sb[:, et, skt * 128:(skt + 1) * 128],
                            qT_sb[:, et, cs],
                            start=(half == 0), stop=(half == 1))
                    e_t = e_p.tile([128, CH], BF16, tag="ep",
                                   name=f"et{c}_{hh}_{skt}")
                    nc.scalar.activation(out=e_t, in_=sc_ps,
                                         func=mybir.ActivationFunctionType.Exp,
                                         bias=0.0,
                                         scale=rk_sb[:, hh, skt:skt + 1])
                    rel = skt - 4 * c
                    if rel >= 0:
                        nc.vector.tensor_mul(out=e_t, in0=e_t,
                                             in1=msk_sb[:, rel, :])
                    e_ts[hh] = e_t
                if filler is not None and skt % 2 == 1:
                    for fn in next(filler, []) or []:
                        fn()
                for hh in range(2):
                    e_t = e_ts[hh]
                    for half in range(2):
                        nc.tensor.matmul(
                            av_ps[hh, half],
                            v_sb[:, skt, hh * 256 + half * 128:
                                 hh * 256 + (half + 1) * 128],
                            e_t, start=first, stop=last)
                    for j in range(4):
                        nc.tensor.matmul(
                            den_ps[hh][:, j:j + 1],
                            e_t[:, j * 128:(j + 1) * 128], ones_sb,
                            start=(first and j == 0), stop=(last and j == 3))
            for hh in range(2):
                dinv = nr_p.tile([128, 4], F32, tag="dinv",
                                 name=f"dinv{c}_{hh}")
                nc.vector.reciprocal(out=dinv, in_=den_ps[hh])
                t_row = col_to_row(dinv, 4)
                rd_t = rd_p.tile([128, CH], F32, tag="rd",
                                 name=f"rd{c}_{hh}")
                for j in range(4):
                    nc.gpsimd.partition_broadcast(
                        rd_t[:, j * 128:(j + 1) * 128],
                        t_row[0:1, j * 128:(j + 1) * 128])
                for half in range(2):
                    nc.vector.tensor_mul(out=aT_t[:, 2 * hh + half, :],
                                         in0=av_ps[hh, half], in1=rd_t)

        def c_burst(c, aT_t, dout):
            cs = slice(c * CH, (c + 1) * CH)
            o_ps = ps_big.tile([128, CH], F32, tag="big",
                               name=f"ops{c}_{dout}")
            for et in range(ET):
                nc.tensor.matmul(
                    o_ps, wo_sb[:, et, dout * 128:(dout + 1) * 128],
                    aT_t[:, et, :],
                    start=(et == 0), stop=(et == ET - 1))
            o_t = o_p.tile([128, CH], F32, tag="op", name=f"ot{c}_{dout}")
            if dout % 2 == 0:
                nc.vector.tensor_copy(out=o_t, in_=o_ps)
            else:
                nc.scalar.copy(out=o_t, in_=o_ps)
            nc.sync.dma_start(
                out=out_d[dout * 128:(dout + 1) * 128, cs], in_=o_t)

        def phase_c(c, aT_t):
            for dout in range(DT):
                c_burst(c, aT_t, dout)

        def c_filler(c, aT_t):
            for dout in range(DT):
                yield [lambda c=c, a=aT_t, d=dout: c_burst(c, a, d)]

        # ---- schedule ----
        aT = {}
        phase_a(0, first=True)
        phase_a(1)
        aT[0] = at_p.tile([128, ET, CH], BF16, tag="atp", name="aT0")
        phase_b_pair(0, aT[0])
        phase_a(2)
        aT[1] = at_p.tile([128, ET, CH], BF16, tag="atp", name="aT1")
        phase_b_pair(1, aT[1])
        phase_a(3)
        aT[2] = at_p.tile([128, ET, CH], BF16, tag="atp", name="aT2")
        phase_b_pair(2, aT[2], filler=c_filler(0, aT[0]))
        phase_c(0, aT[0])
        aT[3] = at_p.tile([128, ET, CH], BF16, tag="atp", name="aT3")
        phase_b_pair(3, aT[3], filler=c_filler(1, aT[1]))
        phase_c(1, aT[1])
        phase_c(2, aT[2])
        phase_c(3, aT[3])


# revision 23
# speedup vs baseline: 1.0443x; 1.0443x over previous
"""Multi-head causal attention (RoPE + QK-RMSNorm) on 8 TRN2 NeuronCores.

Sharding: data parallel on batch (2) x tensor parallel on heads (4 groups of
2 heads).  core = 4*b + g computes, for batch b, heads [2g, 2g+1]:
  q/k/v projections (E-sliced), qk-rmsnorm + rope, causal attention, and the
  Wo partial product over its E slice.  Host sums the 4 partials per batch.

Everything on device runs in the "transposed" orientation:
  qT/kT [e, s], v [s, e], scoresT [sk, sq], out_T [d_out, s]
so no on-chip data transposes are needed; softmax denominators and rms sums
are computed with ones-matmuls on the TensorEngine in *column* layout
[128 positions, few cols].  1/x uses the DVE divide unit; 1/sqrt(x) uses a
sqrt bit-hack seed + 2 Babylonian iterations + reciprocal, all on DVE, so
the ScalarEngine only ever needs the exp table set (no table thrashing).
Per-position column scales are turned into [1, n] rows via a small PE
transpose and broadcast across partitions with gpsimd.partition_broadcast.
"""

import math

import numpy as np
import ml_dtypes

import concourse.bass as bass
import concourse.tile as tile
from concourse import bacc, mybir
from concourse.bass_utils import run_bass_kernel_spmd

# Problem shapes (hardcoded per instructions).
B = 2
S = 2048
D = 2048
H = 8
HD = 256
HALF = 128
EL = 512          # E columns per core (2 heads)
CH = 512          # sq chunk size
NCH = S // CH     # 4
DT = D // 128     # 16 k-tiles over D
ET = EL // 128    # 4 e-tiles
ST = S // 128     # 16 s-tiles
EPS = 1e-6
N_CORES = 8

BF16 = mybir.dt.bfloat16
F32 = mybir.dt.float32
I32 = mybir.dt.int32
NBF = ml_dtypes.bfloat16

SQRT_MAGIC = 0x1FBD1DF5     # sqrt(x) seed: (bits(x) >> 1) + MAGIC

_CACHE: dict = {}


def _build(reps: int = 1):
    nc = bacc.Bacc("TRN2", target_bir_lowering=False, debug=False,
                   num_devices=N_CORES)

    xT_d = nc.dram_tensor("xT", [D, S], BF16, kind="ExternalInput").ap()
    wq_d = nc.dram_tensor("wqT", [D, EL], BF16, kind="ExternalInput").ap()
    wk_d = nc.dram_tensor("wkT", [D, EL], BF16, kind="ExternalInput").ap()
    wv_d = nc.dram_tensor("wvT", [D, EL], BF16, kind="ExternalInput").ap()
    wo_d = nc.dram_tensor("woT", [EL, D], BF16, kind="ExternalInput").ap()
    rtq_d = nc.dram_tensor("rtq", [4, HALF, S], BF16, kind="ExternalInput").ap()
    rtk_d = nc.dram_tensor("rtk", [4, HALF, S], BF16, kind="ExternalInput").ap()
    msk_d = nc.dram_tensor("masks", [4, HALF, CH], BF16, kind="ExternalInput").ap()
    out_d = nc.dram_tensor("outT", [D, S], F32, kind="ExternalOutput").ap()

    with tile.TileContext(nc) as tc:
        for _ in range(reps):
            _emit(tc, nc, xT_d, wq_d, wk_d, wv_d, wo_d, rtq_d, rtk_d, msk_d,
                  out_d)
    nc.compile()
    return nc


def _emit(tc, nc, xT_d, wq_d, wk_d, wv_d, wo_d, rtq_d, rtk_d, msk_d, out_d):
    from contextlib import ExitStack
    ctx = ExitStack()
    with ctx:
        persist = ctx.enter_context(tc.tile_pool(name="persist", bufs=1))
        xs_p = ctx.enter_context(tc.tile_pool(name="xs", bufs=18))
        rt_p = ctx.enter_context(tc.tile_pool(name="rt", bufs=1))
        sq_p = ctx.enter_context(tc.tile_pool(name="sq", bufs=5))
        qr_p = ctx.enter_context(tc.tile_pool(name="qr", bufs=6))
        qs_p = ctx.enter_context(tc.tile_pool(name="qs", bufs=4))
        rtmp_p = ctx.enter_context(tc.tile_pool(name="rtmp", bufs=6))
        rb_p = ctx.enter_context(tc.tile_pool(name="rb", bufs=3))
        rd_p = ctx.enter_context(tc.tile_pool(name="rd", bufs=2))
        e_p = ctx.enter_context(tc.tile_pool(name="ep", bufs=8))
        o_p = ctx.enter_context(tc.tile_pool(name="op", bufs=3))
        nr_p = ctx.enter_context(tc.tile_pool(name="nrp", bufs=3))
        at_p = ctx.enter_context(tc.tile_pool(name="atp", bufs=2))

        ps_big = ctx.enter_context(tc.tile_pool(name="psb", bufs=6, space="PSUM"))
        ps_col = ctx.enter_context(tc.tile_pool(name="psc", bufs=2, space="PSUM"))

        # ---- persistent tiles ----
        wq_sb = persist.tile([128, DT, EL], BF16, tag="wq")
        wk_sb = persist.tile([128, DT, EL], BF16, tag="wk")
        wv_sb = persist.tile([128, DT, EL], BF16, tag="wv")
        wo_sb = persist.tile([128, ET, D], BF16, tag="wo")
        qT_sb = persist.tile([128, ET, S], BF16, tag="qT")
        kT_sb = persist.tile([128, ET, S], BF16, tag="kT")
        v_sb = persist.tile([128, ST, EL], BF16, tag="v")
        msk_sb = persist.tile([128, 4, CH], BF16, tag="msk")
        ones_sb = persist.tile([128, 1], BF16, tag="ones")
        rk_sb = persist.tile([128, 2, ST], F32, tag="rk")   # f_k per (head, sk)
        nc.vector.memset(ones_sb, 1.0)

        nc.sync.dma_start(out=wq_sb, in_=wq_d.rearrange("(t p) e -> p t e", p=128))

        def load_wk():
            nc.sync.dma_start(out=wk_sb, in_=wk_d.rearrange("(t p) e -> p t e", p=128))

        def load_wv():
            nc.sync.dma_start(out=wv_sb, in_=wv_d.rearrange("(t p) e -> p t e", p=128))

        def load_rest():
            nc.sync.dma_start(out=wo_sb, in_=wo_d.rearrange("(t p) d -> p t d", p=128))
            nc.sync.dma_start(out=msk_sb, in_=msk_d.rearrange("t p s -> p t s"))

        LN_EPS = float(HD * EPS)

        def nr_rsqrt(dst, src_ps, n, scale16):
            """dst[128, n] = (src_ps + 256*eps)^(-1/2) (*16), DVE only."""
            x = nr_p.tile([128, 8], F32, tag="nrx", name=f"nrx{nr_rsqrt.i}")[:, :n]
            nc.vector.tensor_scalar(out=x, in0=src_ps, scalar1=LN_EPS,
                                    scalar2=None, op0=mybir.AluOpType.add)
            s = nr_p.tile([128, 8], F32, tag="nry", name=f"nry{nr_rsqrt.i}")[:, :n]
            nc.vector.tensor_scalar(
                out=s.bitcast(I32), in0=x.bitcast(I32), scalar1=1,
                scalar2=None, op0=mybir.AluOpType.arith_shift_right)
            nc.vector.tensor_scalar(
                out=s.bitcast(I32), in0=s.bitcast(I32), scalar1=SQRT_MAGIC,
                scalar2=None, op0=mybir.AluOpType.add)
            for it in range(2):
                r = nr_p.tile([128, 8], F32, tag="nrt",
                              name=f"nrt{nr_rsqrt.i}_{it}")[:, :n]
                nc.vector.reciprocal(out=r, in_=s)
                nc.vector.tensor_mul(out=r, in0=r, in1=x)     # x / s
                nc.vector.tensor_add(out=s, in0=s, in1=r)
                nc.vector.tensor_scalar(out=s, in0=s, scalar1=0.5,
                                        scalar2=None, op0=mybir.AluOpType.mult)
            nr_rsqrt.i += 1
            nc.vector.reciprocal(out=dst, in_=s)
            if scale16:
                nc.vector.tensor_scalar(out=dst, in0=dst, scalar1=16.0,
                                        scalar2=None, op0=mybir.AluOpType.mult)

        nr_rsqrt.i = 0

        def col_to_row(src, n):
            """[128, n] f32 cols -> [1, n*128] row on partition 0 (one DMA).

            row[0, 128*j + p] = src[p, j]"""
            row = nr_p.tile([1, 1024], F32, tag="row",
                            name=f"row{col_to_row.i}")
            col_to_row.i += 1
            for j in range(n):
                nc.gpsimd.dma_start(
                    out=row[:, j * 128:(j + 1) * 128], in_=src[:, j:j + 1])
            return row
        col_to_row.i = 0

        def _rope(dst_sb, et0, cs, x1, x2, tab):
            """dst[:, et0, cs]   = x1*tab[0] - x2*tab[3]   (cw1, sw2)
               dst[:, et0+1, cs] = x2*tab[2] + x1*tab[1]   (cw2, sw1)"""
            t1 = rtmp_p.tile([128, CH], BF16, tag="rtmp")
            t2 = rtmp_p.tile([128, CH], BF16, tag="rtmp")
            nc.vector.tensor_mul(out=t1, in0=x1, in1=tab[:, 0, :])
            nc.vector.tensor_mul(out=t2, in0=x2, in1=tab[:, 3, :])
            nc.vector.tensor_sub(out=dst_sb[:, et0, cs], in0=t1, in1=t2)
            t3 = rtmp_p.tile([128, CH], BF16, tag="rtmp")
            t4 = rtmp_p.tile([128, CH], BF16, tag="rtmp")
            nc.vector.tensor_mul(out=t3, in0=x2, in1=tab[:, 2, :])
            nc.vector.tensor_mul(out=t4, in0=x1, in1=tab[:, 1, :])
            nc.vector.tensor_add(out=dst_sb[:, et0 + 1, cs], in0=t3, in1=t4)

        def phase_a(c, first=False):
            """QKV projections + rmsnorm + rope for chunk c."""
            cs = slice(c * CH, (c + 1) * CH)
            xs = []
            for dt in range(DT):
                t = xs_p.tile([128, CH], BF16, tag="xs")
                nc.sync.dma_start(
                    out=t, in_=xT_d[dt * 128:(dt + 1) * 128, cs])
                xs.append(t)
            rtq_t = rt_p.tile([128, 4, CH], BF16, tag="rtq")
            nc.sync.dma_start(out=rtq_t,
                              in_=rtq_d[:, :, cs].rearrange("t p s -> p t s"))
            rtk_t = rt_p.tile([128, 4, CH], BF16, tag="rtk")
            nc.sync.dma_start(out=rtk_t,
                              in_=rtk_d[:, :, cs].rearrange("t p s -> p t s"))
            if first:
                load_wk()

            # ---------- q ----------
            rq_ps = ps_col.tile([128, 8], F32, tag="col")
            qr = []
            sqs = []
            for et in range(ET):
                q_ps = ps_big.tile([128, CH], F32, tag="big")
                for dt in range(DT):
                    nc.tensor.matmul(
                        q_ps, wq_sb[:, dt, et * 128:(et + 1) * 128], xs[dt],
                        start=(dt == 0), stop=(dt == DT - 1))
                t = qr_p.tile([128, CH], BF16, tag="qr")
                nc.scalar.copy(out=t, in_=q_ps)
                qr.append(t)
                sqt = sq_p.tile([128, CH], BF16, tag="sq")
                nc.vector.tensor_mul(out=sqt, in0=t, in1=t)
                sqs.append(sqt)
            for et in range(ET):
                hh = et // 2
                for j in range(4):
                    nc.tensor.matmul(
                        rq_ps[:, 4 * hh + j: 4 * hh + j + 1],
                        sqs[et][:, j * 128:(j + 1) * 128], ones_sb,
                        start=(et == 0 and j == 0),
                        stop=(et == ET - 1 and j == 3))
            rq_sb = nr_p.tile([128, 8], F32, tag="rq")
            nr_rsqrt(rq_sb, rq_ps, 8, scale16=False)
            t_row = col_to_row(rq_sb, 8)
            rb = []
            for hh in range(2):
                rbt = rb_p.tile([128, CH], F32, tag="rb")
                nc.gpsimd.partition_broadcast(
                    rbt, t_row[0:1, hh * CH:(hh + 1) * CH])
                rb.append(rbt)
            if first:
                load_wv()
            for hh in range(2):
                q1 = qs_p.tile([128, CH], BF16, tag="qs")
                nc.vector.tensor_mul(out=q1, in0=qr[2 * hh], in1=rb[hh])
                q2 = qs_p.tile([128, CH], BF16, tag="qs")
                nc.vector.tensor_mul(out=q2, in0=qr[2 * hh + 1], in1=rb[hh])
                _rope(qT_sb, 2 * hh, cs, q1, q2, rtq_t)

            # ---------- k ----------
            rk_ps = ps_col.tile([128, 8], F32, tag="col")
            kr = []
            ksqs = []
            for et in range(ET):
                k_ps = ps_big.tile([128, CH], F32, tag="big")
                for dt in range(DT):
                    nc.tensor.matmul(
                        k_ps, wk_sb[:, dt, et * 128:(et + 1) * 128], xs[dt],
                        start=(dt == 0), stop=(dt == DT - 1))
                t = qr_p.tile([128, CH], BF16, tag="qr")
                nc.scalar.copy(out=t, in_=k_ps)
                kr.append(t)
                sqt = sq_p.tile([128, CH], BF16, tag="sq")
                nc.vector.tensor_mul(out=sqt, in0=t, in1=t)
                ksqs.append(sqt)
            for et in range(ET):
                hh = et // 2
                for j in range(4):
                    nc.tensor.matmul(
                        rk_ps[:, 4 * hh + j: 4 * hh + j + 1],
                        ksqs[et][:, j * 128:(j + 1) * 128], ones_sb,
                        start=(et == 0 and j == 0),
                        stop=(et == ET - 1 and j == 3))
            nr_rsqrt(rk_sb[:, 0, 4 * c:4 * c + 4], rk_ps[:, 0:4], 4,
                     scale16=True)
            nr_rsqrt(rk_sb[:, 1, 4 * c:4 * c + 4], rk_ps[:, 4:8], 4,
                     scale16=True)
            for hh in range(2):
                _rope(kT_sb, 2 * hh, cs, kr[2 * hh], kr[2 * hh + 1], rtk_t)

            # ---------- v ----------
            for st in range(4):
                v_ps = ps_big.tile([128, EL], F32, tag="big")
                for dt in range(DT):
                    nc.tensor.matmul(
                        v_ps, xs[dt][:, st * 128:(st + 1) * 128],
                        wv_sb[:, dt, :],
                        start=(dt == 0), stop=(dt == DT - 1))
                nc.scalar.copy(out=v_sb[:, 4 * c + st, :], in_=v_ps)
            if first:
                load_rest()

        def phase_b_pair(c, aT_t, filler=None):
            """Attention for chunk c, both local heads, skt-interleaved.

            filler: optional iterator yielding callables that emit a burst of
            independent PE work (used to keep the PE fed during the serial
            score->exp->mask->av chains)."""
            cs = slice(c * CH, (c + 1) * CH)
            n_sk = 4 * c + 4
            av_ps = {(hh, i): ps_big.tile([128, CH], F32, tag="big",
                                          name=f"av{c}_{hh}_{i}")
                     for hh in range(2) for i in range(2)}
            den_ps = {hh: ps_col.tile([128, 4], F32, tag="col",
                                      name=f"den{c}_{hh}")
                      for hh in range(2)}
            for skt in range(n_sk):
                first, last = (skt == 0), (skt == n_sk - 1)
                e_ts = {}
                for hh in range(2):
                    sc_ps = ps_big.tile([128, CH], F32, tag="big",
                                        name=f"sc{c}_{hh}_{skt}")
                    for half in range(2):
                        et = 2 * hh + half
                        nc.tensor.matmul(
                            sc_ps, kT_sb[:, et, skt * 128:(skt + 1) * 128],
                            qT_sb[:, et, cs],
                            start=(half == 0), stop=(half == 1))
                    e_t = e_p.tile([128, CH], BF16, tag="ep",
                                   name=f"et{c}_{hh}_{skt}")
                    nc.scalar.activation(out=e_t, in_=sc_ps,
                                         func=mybir.ActivationFunctionType.Exp,
                                         bias=0.0,
                                         scale=rk_sb[:, hh, skt:skt + 1])
                    rel = skt - 4 * c
                    if rel >= 0:
                        nc.vector.tensor_mul(out=e_t, in0=e_t,
                                             in1=msk_sb[:, rel, :])
                    e_ts[hh] = e_t
                if filler is not None and skt % 2 == 1:
                    for fn in next(filler, []) or []:
                        fn()
                for hh in range(2):
                    e_t = e_ts[hh]
                    for half in range(2):
                        nc.tensor.matmul(
                            av_ps[hh, half],
                            v_sb[:, skt, hh * 256 + half * 128:
                                 hh * 256 + (half + 1) * 128],
                            e_t, start=first, stop=last)
                    for j in range(4):
                        nc.tensor.matmul(
                            den_ps[hh][:, j:j + 1],
                            e_t[:, j * 128:(j + 1) * 128], ones_sb,
                            start=(first and j == 0), stop=(last and j == 3))
            for hh in range(2):
                dinv = nr_p.tile([128, 4], F32, tag="dinv",
                                 name=f"dinv{c}_{hh}")
                nc.vector.reciprocal(out=dinv, in_=den_ps[hh])
                t_row = col_to_row(dinv, 4)
                rd_t = rd_p.tile([128, CH], F32, tag="rd",
                                 name=f"rd{c}_{hh}")
                for j in range(4):
                    nc.gpsimd.partition_broadcast(
                        rd_t[:, j * 128:(j + 1) * 128],
                        t_row[0:1, j * 128:(j + 1) * 128])
                for half in range(2):
                    nc.vector.tensor_mul(out=aT_t[:, 2 * hh + half, :],
                                         in0=av_ps[hh, half], in1=rd_t)

        def c_burst(c, aT_t, dout):
            cs = slice(c * CH, (c + 1) * CH)
            o_ps = ps_big.tile([128, CH], F32, tag="big",
                               name=f"ops{c}_{dout}")
            for et in range(ET):
                nc.tensor.matmul(
                    o_ps, wo_sb[:, et, dout * 128:(dout + 1) * 128],
                    aT_t[:, et, :],
                    start=(et == 0), stop=(et == ET - 1))
            o_t = o_p.tile([128, CH], F32, tag="op", name=f"ot{c}_{dout}")
            if dout % 2 == 0:
                nc.vector.tensor_copy(out=o_t, in_=o_ps)
            else:
                nc.scalar.copy(out=o_t, in_=o_ps)
            nc.sync.dma_start(
                out=out_d[dout * 128:(dout + 1) * 128, cs], in_=o_t)

        def phase_c(c, aT_t):
            for dout in range(DT):
                c_burst(c, aT_t, dout)

        def c_filler(c, aT_t):
            for dout in range(DT):
                yield [lambda c=c, a=aT_t, d=dout: c_burst(c, a, d)]

        # ---- schedule ----
        aT = {}
        phase_a(0, first=True)
        phase_a(1)
        aT[0] = at_p.tile([128, ET, CH], BF16, tag="atp", name="aT0")
        phase_b_pair(0, aT[0])
        phase_a(2)
        aT[1] = at_p.tile([128, ET, CH], BF16, tag="atp", name="aT1")
        phase_b_pair(1, aT[1])
        phase_a(3)
        aT[2] = at_p.tile([128, ET, CH], BF16, tag="atp", name="aT2")
        phase_b_pair(2, aT[2], filler=c_filler(0, aT[0]))
        phase_c(0, aT[0])
        aT[3] = at_p.tile([128, ET, CH], BF16, tag="atp", name="aT3")
        phase_b_pair(3, aT[3], filler=c_filler(1, aT[1]))
        phase_c(1, aT[1])
        phase_c(2, aT[2])
        phase_c(3, aT[3])


# revision 24
# speedup vs baseline: 1.1024x; 1.0556x over previous
"""Multi-head causal attention (RoPE + QK-RMSNorm) on 8 TRN2 NeuronCores.

Sharding: data parallel on batch (2) x tensor parallel on heads (4 groups of
2 heads).  core = 4*b + g computes, for batch b, heads [2g, 2g+1]:
  q/k/v projections (E-sliced), qk-rmsnorm + rope, causal attention, and the
  Wo partial product over its E slice.  Host sums the 4 partials per batch.

Everything on device runs in the "transposed" orientation:
  qT/kT [e, s], v [s, e], scoresT [sk, sq], out_T [d_out, s]
so no on-chip data transposes are needed; softmax denominators and rms sums
are computed with ones-matmuls on the TensorEngine in *column* layout
[128 positions, few cols].  1/x uses the DVE divide unit; 1/sqrt(x) uses a
sqrt bit-hack seed + 2 Babylonian iterations + reciprocal, all on DVE, so
the ScalarEngine only ever needs the exp table set (no table thrashing).
Per-position column scales are turned into [1, n] rows via a small PE
transpose and broadcast across partitions with gpsimd.partition_broadcast.
"""

import math

import numpy as np
import ml_dtypes

import concourse.bass as bass
import concourse.tile as tile
from concourse import bacc, mybir
from concourse.bass_utils import run_bass_kernel_spmd

# Problem shapes (hardcoded per instructions).
B = 2
S = 2048
D = 2048
H = 8
HD = 256
HALF = 128
EL = 512          # E columns per core (2 heads)
CH = 512          # sq chunk size
NCH = S // CH     # 4
DT = D // 128     # 16 k-tiles over D
ET = EL // 128    # 4 e-tiles
ST = S // 128     # 16 s-tiles
EPS = 1e-6
N_CORES = 8

BF16 = mybir.dt.bfloat16
F32 = mybir.dt.float32
I32 = mybir.dt.int32
NBF = ml_dtypes.bfloat16

SQRT_MAGIC = 0x1FBD1DF5     # sqrt(x) seed: (bits(x) >> 1) + MAGIC

_CACHE: dict = {}


def _build(reps: int = 1):
    nc = bacc.Bacc("TRN2", target_bir_lowering=False, debug=False,
                   num_devices=N_CORES)

    xT_d = nc.dram_tensor("xT", [D, S], BF16, kind="ExternalInput").ap()
    wq_d = nc.dram_tensor("wqT", [D, EL], BF16, kind="ExternalInput").ap()
    wk_d = nc.dram_tensor("wkT", [D, EL], BF16, kind="ExternalInput").ap()
    wv_d = nc.dram_tensor("wvT", [D, EL], BF16, kind="ExternalInput").ap()
    wo_d = nc.dram_tensor("woT", [EL, D], BF16, kind="ExternalInput").ap()
    rtq_d = nc.dram_tensor("rtq", [4, HALF, S], BF16, kind="ExternalInput").ap()
    rtk_d = nc.dram_tensor("rtk", [4, HALF, S], BF16, kind="ExternalInput").ap()
    msk_d = nc.dram_tensor("masks", [4, HALF, CH], BF16, kind="ExternalInput").ap()
    out_d = nc.dram_tensor("outT", [D, S], F32, kind="ExternalOutput").ap()

    with tile.TileContext(nc) as tc:
        for _ in range(reps):
            _emit(tc, nc, xT_d, wq_d, wk_d, wv_d, wo_d, rtq_d, rtk_d, msk_d,
                  out_d)
    nc.compile()
    return nc


def _emit(tc, nc, xT_d, wq_d, wk_d, wv_d, wo_d, rtq_d, rtk_d, msk_d, out_d):
    from contextlib import ExitStack
    ctx = ExitStack()
    with ctx:
        persist = ctx.enter_context(tc.tile_pool(name="persist", bufs=1))
        xs_p = ctx.enter_context(tc.tile_pool(name="xs", bufs=18))
        rt_p = ctx.enter_context(tc.tile_pool(name="rt", bufs=1))
        sq_p = ctx.enter_context(tc.tile_pool(name="sq", bufs=4))
        qr_p = ctx.enter_context(tc.tile_pool(name="qr", bufs=9))
        qs_p = ctx.enter_context(tc.tile_pool(name="qs", bufs=4))
        rtmp_p = ctx.enter_context(tc.tile_pool(name="rtmp", bufs=6))
        rb_p = ctx.enter_context(tc.tile_pool(name="rb", bufs=2))
        rd_p = ctx.enter_context(tc.tile_pool(name="rd", bufs=2))
        e_p = ctx.enter_context(tc.tile_pool(name="ep", bufs=8))
        o_p = ctx.enter_context(tc.tile_pool(name="op", bufs=3))
        nr_p = ctx.enter_context(tc.tile_pool(name="nrp", bufs=3))
        at_p = ctx.enter_context(tc.tile_pool(name="atp", bufs=2))

        ps_big = ctx.enter_context(tc.tile_pool(name="psb", bufs=6, space="PSUM"))
        ps_col = ctx.enter_context(tc.tile_pool(name="psc", bufs=2, space="PSUM"))

        # ---- persistent tiles ----
        wq_sb = persist.tile([128, DT, EL], BF16, tag="wq")
        wk_sb = persist.tile([128, DT, EL], BF16, tag="wk")
        wv_sb = persist.tile([128, DT, EL], BF16, tag="wv")
        wo_sb = persist.tile([128, ET, D], BF16, tag="wo")
        qT_sb = persist.tile([128, ET, S], BF16, tag="qT")
        kT_sb = persist.tile([128, ET, S], BF16, tag="kT")
        v_sb = persist.tile([128, ST, EL], BF16, tag="v")
        msk_sb = persist.tile([128, 4, CH], BF16, tag="msk")
        ones_sb = persist.tile([128, 1], BF16, tag="ones")
        rk_sb = persist.tile([128, 2, ST], F32, tag="rk")   # f_k per (head, sk)
        nc.vector.memset(ones_sb, 1.0)

        nc.sync.dma_start(out=wq_sb, in_=wq_d.rearrange("(t p) e -> p t e", p=128))

        def load_wk():
            nc.sync.dma_start(out=wk_sb, in_=wk_d.rearrange("(t p) e -> p t e", p=128))

        def load_wv():
            nc.sync.dma_start(out=wv_sb, in_=wv_d.rearrange("(t p) e -> p t e", p=128))

        def load_rest():
            nc.sync.dma_start(out=wo_sb, in_=wo_d.rearrange("(t p) d -> p t d", p=128))
            nc.sync.dma_start(out=msk_sb, in_=msk_d.rearrange("t p s -> p t s"))

        LN_EPS = float(HD * EPS)

        def nr_rsqrt(dst, src_ps, n, scale16):
            """dst[128, n] = (src_ps + 256*eps)^(-1/2) (*16), DVE only."""
            x = nr_p.tile([128, 8], F32, tag="nrx", name=f"nrx{nr_rsqrt.i}")[:, :n]
            nc.vector.tensor_scalar(out=x, in0=src_ps, scalar1=LN_EPS,
                                    scalar2=None, op0=mybir.AluOpType.add)
            s = nr_p.tile([128, 8], F32, tag="nry", name=f"nry{nr_rsqrt.i}")[:, :n]
            nc.vector.tensor_scalar(
                out=s.bitcast(I32), in0=x.bitcast(I32), scalar1=1,
                scalar2=None, op0=mybir.AluOpType.arith_shift_right)
            nc.vector.tensor_scalar(
                out=s.bitcast(I32), in0=s.bitcast(I32), scalar1=SQRT_MAGIC,
                scalar2=None, op0=mybir.AluOpType.add)
            for it in range(2):
                r = nr_p.tile([128, 8], F32, tag="nrt",
                              name=f"nrt{nr_rsqrt.i}_{it}")[:, :n]
                nc.vector.reciprocal(out=r, in_=s)
                nc.vector.tensor_mul(out=r, in0=r, in1=x)     # x / s
                nc.vector.tensor_add(out=s, in0=s, in1=r)
                nc.vector.tensor_scalar(out=s, in0=s, scalar1=0.5,
                                        scalar2=None, op0=mybir.AluOpType.mult)
            nr_rsqrt.i += 1
            nc.vector.reciprocal(out=dst, in_=s)
            if scale16:
                nc.vector.tensor_scalar(out=dst, in0=dst, scalar1=16.0,
                                        scalar2=None, op0=mybir.AluOpType.mult)

        nr_rsqrt.i = 0

        def col_to_row(src, n):
            """[128, n] f32 cols -> [1, n*128] row on partition 0 (one DMA).

            row[0, 128*j + p] = src[p, j]"""
            row = nr_p.tile([1, 1024], F32, tag="row",
                            name=f"row{col_to_row.i}")
            col_to_row.i += 1
            for j in range(n):
                nc.gpsimd.dma_start(
                    out=row[:, j * 128:(j + 1) * 128], in_=src[:, j:j + 1])
            return row
        col_to_row.i = 0

        def _rope(dst_sb, et0, cs, x1, x2, tab):
            """dst[:, et0, cs]   = x1*tab[0] - x2*tab[3]   (cw1, sw2)
               dst[:, et0+1, cs] = x2*tab[2] + x1*tab[1]   (cw2, sw1)"""
            t1 = rtmp_p.tile([128, CH], BF16, tag="rtmp")
            t2 = rtmp_p.tile([128, CH], BF16, tag="rtmp")
            nc.vector.tensor_mul(out=t1, in0=x1, in1=tab[:, 0, :])
            nc.vector.tensor_mul(out=t2, in0=x2, in1=tab[:, 3, :])
            nc.vector.tensor_sub(out=dst_sb[:, et0, cs], in0=t1, in1=t2)
            t3 = rtmp_p.tile([128, CH], BF16, tag="rtmp")
            t4 = rtmp_p.tile([128, CH], BF16, tag="rtmp")
            nc.vector.tensor_mul(out=t3, in0=x2, in1=tab[:, 2, :])
            nc.vector.tensor_mul(out=t4, in0=x1, in1=tab[:, 1, :])
            nc.vector.tensor_add(out=dst_sb[:, et0 + 1, cs], in0=t3, in1=t4)

        def phase_a(c, first=False):
            """QKV projections + rmsnorm + rope for chunk c."""
            cs = slice(c * CH, (c + 1) * CH)
            xs = []
            for dt in range(DT):
                t = xs_p.tile([128, CH], BF16, tag="xs")
                nc.sync.dma_start(
                    out=t, in_=xT_d[dt * 128:(dt + 1) * 128, cs])
                xs.append(t)
            rtq_t = rt_p.tile([128, 4, CH], BF16, tag="rtq")
            nc.sync.dma_start(out=rtq_t,
                              in_=rtq_d[:, :, cs].rearrange("t p s -> p t s"))
            rtk_t = rt_p.tile([128, 4, CH], BF16, tag="rtk")
            nc.sync.dma_start(out=rtk_t,
                              in_=rtk_d[:, :, cs].rearrange("t p s -> p t s"))
            if first:
                load_wk()

            # ---------- q ----------
            rq_ps = ps_col.tile([128, 8], F32, tag="col")
            qr = []
            sqs = []
            for et in range(ET):
                q_ps = ps_big.tile([128, CH], F32, tag="big")
                for dt in range(DT):
                    nc.tensor.matmul(
                        q_ps, wq_sb[:, dt, et * 128:(et + 1) * 128], xs[dt],
                        start=(dt == 0), stop=(dt == DT - 1))
                t = qr_p.tile([128, CH], BF16, tag="qr")
                nc.scalar.copy(out=t, in_=q_ps)
                qr.append(t)
                sqt = sq_p.tile([128, CH], BF16, tag="sq")
                nc.vector.tensor_mul(out=sqt, in0=t, in1=t)
                sqs.append(sqt)
            for et in range(ET):
                hh = et // 2
                for j in range(4):
                    nc.tensor.matmul(
                        rq_ps[:, 4 * hh + j: 4 * hh + j + 1],
                        sqs[et][:, j * 128:(j + 1) * 128], ones_sb,
                        start=(et == 0 and j == 0),
                        stop=(et == ET - 1 and j == 3))
            def q_tail():
                rq_sb = nr_p.tile([128, 8], F32, tag="rq")
                nr_rsqrt(rq_sb, rq_ps, 8, scale16=False)
                t_row = col_to_row(rq_sb, 8)
                for hh in range(2):
                    rbt = rb_p.tile([128, CH], F32, tag="rb")
                    nc.gpsimd.partition_broadcast(
                        rbt, t_row[0:1, hh * CH:(hh + 1) * CH])
                    q1 = qs_p.tile([128, CH], BF16, tag="qs")
                    nc.vector.tensor_mul(out=q1, in0=qr[2 * hh], in1=rbt)
                    q2 = qs_p.tile([128, CH], BF16, tag="qs")
                    nc.vector.tensor_mul(out=q2, in0=qr[2 * hh + 1], in1=rbt)
                    _rope(qT_sb, 2 * hh, cs, q1, q2, rtq_t)
            if first:
                load_wv()

            # ---------- k ----------
            rk_ps = ps_col.tile([128, 8], F32, tag="col")
            kr = []
            ksqs = []
            for et in range(ET):
                k_ps = ps_big.tile([128, CH], F32, tag="big")
                for dt in range(DT):
                    nc.tensor.matmul(
                        k_ps, wk_sb[:, dt, et * 128:(et + 1) * 128], xs[dt],
                        start=(dt == 0), stop=(dt == DT - 1))
                t = qr_p.tile([128, CH], BF16, tag="qr")
                nc.scalar.copy(out=t, in_=k_ps)
                kr.append(t)
                sqt = sq_p.tile([128, CH], BF16, tag="sq")
                nc.vector.tensor_mul(out=sqt, in0=t, in1=t)
                ksqs.append(sqt)
            for et in range(ET):
                hh = et // 2
                for j in range(4):
                    nc.tensor.matmul(
                        rk_ps[:, 4 * hh + j: 4 * hh + j + 1],
                        ksqs[et][:, j * 128:(j + 1) * 128], ones_sb,
                        start=(et == 0 and j == 0),
                        stop=(et == ET - 1 and j == 3))
            q_tail()

            def k_tail():
                nr_rsqrt(rk_sb[:, 0, 4 * c:4 * c + 4], rk_ps[:, 0:4], 4,
                         scale16=True)
                nr_rsqrt(rk_sb[:, 1, 4 * c:4 * c + 4], rk_ps[:, 4:8], 4,
                         scale16=True)
                for hh in range(2):
                    _rope(kT_sb, 2 * hh, cs, kr[2 * hh], kr[2 * hh + 1],
                          rtk_t)

            # ---------- v ----------
            for st in range(4):
                v_ps = ps_big.tile([128, EL], F32, tag="big")
                for dt in range(DT):
                    nc.tensor.matmul(
                        v_ps, xs[dt][:, st * 128:(st + 1) * 128],
                        wv_sb[:, dt, :],
                        start=(dt == 0), stop=(dt == DT - 1))
                nc.scalar.copy(out=v_sb[:, 4 * c + st, :], in_=v_ps)
            k_tail()
            if first:
                load_rest()

        def phase_b_pair(c, aT_t, filler=None):
            """Attention for chunk c, both local heads, skt-interleaved.

            filler: optional iterator yielding callables that emit a burst of
            independent PE work (used to keep the PE fed during the serial
            score->exp->mask->av chains)."""
            cs = slice(c * CH, (c + 1) * CH)
            n_sk = 4 * c + 4
            av_ps = {(hh, i): ps_big.tile([128, CH], F32, tag="big",
                                          name=f"av{c}_{hh}_{i}")
                     for hh in range(2) for i in range(2)}
            den_ps = {hh: ps_col.tile([128, 4], F32, tag="col",
                                      name=f"den{c}_{hh}")
                      for hh in range(2)}
            for skt in range(n_sk):
                first, last = (skt == 0), (skt == n_sk - 1)
                e_ts = {}
                for hh in range(2):
                    sc_ps = ps_big.tile([128, CH], F32, tag="big",
                                        name=f"sc{c}_{hh}_{skt}")
                    for half in range(2):
                        et = 2 * hh + half
                        nc.tensor.matmul(
                            sc_ps, kT_sb[:, et, skt * 128:(skt + 1) * 128],
                            qT_sb[:, et, cs],
                            start=(half == 0), stop=(half == 1))
                    e_t = e_p.tile([128, CH], BF16, tag="ep",
                                   name=f"et{c}_{hh}_{skt}")
                    nc.scalar.activation(out=e_t, in_=sc_ps,
                                         func=mybir.ActivationFunctionType.Exp,
                                         bias=0.0,
                                         scale=rk_sb[:, hh, skt:skt + 1])
                    rel = skt - 4 * c
                    if rel >= 0:
                        nc.vector.tensor_mul(out=e_t, in0=e_t,
                                             in1=msk_sb[:, rel, :])
                    e_ts[hh] = e_t
                if filler is not None and skt % 2 == 1:
                    for fn in next(filler, []) or []:
                        fn()
                for hh in range(2):
                    e_t = e_ts[hh]
                    for half in range(2):
                        nc.tensor.matmul(
                            av_ps[hh, half],
                            v_sb[:, skt, hh * 256 + half * 128:
                                 hh * 256 + (half + 1) * 128],
                            e_t, start=first, stop=last)
                    for j in range(4):
                        nc.tensor.matmul(
                            den_ps[hh][:, j:j + 1],
                            e_t[:, j * 128:(j + 1) * 128], ones_sb,
                            start=(first and j == 0), stop=(last and j == 3))
            for hh in range(2):
                dinv = nr_p.tile([128, 4], F32, tag="dinv",
                                 name=f"dinv{c}_{hh}")
                nc.vector.reciprocal(out=dinv, in_=den_ps[hh])
                t_row = col_to_row(dinv, 4)
                rd_t = rd_p.tile([128, CH], F32, tag="rd",
                                 name=f"rd{c}_{hh}")
                for j in range(4):
                    nc.gpsimd.partition_broadcast(
                        rd_t[:, j * 128:(j + 1) * 128],
                        t_row[0:1, j * 128:(j + 1) * 128])
                for half in range(2):
                    nc.vector.tensor_mul(out=aT_t[:, 2 * hh + half, :],
                                         in0=av_ps[hh, half], in1=rd_t)

        def c_burst(c, aT_t, dout):
            cs = slice(c * CH, (c + 1) * CH)
            o_ps = ps_big.tile([128, CH], F32, tag="big",
                               name=f"ops{c}_{dout}")
            for et in range(ET):
                nc.tensor.matmul(
                    o_ps, wo_sb[:, et, dout * 128:(dout + 1) * 128],
                    aT_t[:, et, :],
                    start=(et == 0), stop=(et == ET - 1))
            o_t = o_p.tile([128, CH], F32, tag="op", name=f"ot{c}_{dout}")
            if dout % 2 == 0:
                nc.vector.tensor_copy(out=o_t, in_=o_ps)
            else:
                nc.scalar.copy(out=o_t, in_=o_ps)
            nc.sync.dma_start(
                out=out_d[dout * 128:(dout + 1) * 128, cs], in_=o_t)

        def phase_c(c, aT_t):
            for dout in range(DT):
                c_burst(c, aT_t, dout)

        def c_filler(c, aT_t):
            for dout in range(DT):
                yield [lambda c=c, a=aT_t, d=dout: c_burst(c, a, d)]

        # ---- schedule ----
        aT = {}
        phase_a(0, first=True)
        phase_a(1)
        aT[0] = at_p.tile([128, ET, CH], BF16, tag="atp", name="aT0")
        phase_b_pair(0, aT[0])
        phase_a(2)
        aT[1] = at_p.tile([128, ET, CH], BF16, tag="atp", name="aT1")
        phase_b_pair(1, aT[1])
        phase_a(3)
        aT[2] = at_p.tile([128, ET, CH], BF16, tag="atp", name="aT2")
        phase_b_pair(2, aT[2], filler=c_filler(0, aT[0]))
        phase_c(0, aT[0])
        aT[3] = at_p.tile([128, ET, CH], BF16, tag="atp", name="aT3")
        phase_b_pair(3, aT[3], filler=c_filler(1, aT[1]))
        phase_c(1, aT[1])
        phase_c(2, aT[2])
        phase_c(3, aT[3])


# revision 25
# speedup vs baseline: 1.1948x; 1.0838x over previous
"""Multi-head causal attention (RoPE + QK-RMSNorm) on 8 TRN2 NeuronCores.

Sharding: data parallel on batch (2) x tensor parallel on heads (4 groups of
2 heads).  core = 4*b + g computes, for batch b, heads [2g, 2g+1]:
  q/k/v projections (E-sliced), qk-rmsnorm + rope, causal attention, and the
  Wo partial product over its E slice.  Host sums the 4 partials per batch.

Everything on device runs in the "transposed" orientation:
  qT/kT [e, s], v [s, e], scoresT [sk, sq], out_T [d_out, s]
so no on-chip data transposes are needed; softmax denominators and rms sums
are computed with ones-matmuls on the TensorEngine in *column* layout
[128 positions, few cols].  1/x uses the DVE divide unit; 1/sqrt(x) uses a
sqrt bit-hack seed + 2 Babylonian iterations + reciprocal, all on DVE, so
the ScalarEngine only ever needs the exp table set (no table thrashing).
Per-position column scales are turned into [1, n] rows via a small PE
transpose and broadcast across partitions with gpsimd.partition_broadcast.
"""

import math

import numpy as np
import ml_dtypes

import concourse.bass as bass
import concourse.tile as tile
from concourse import bacc, mybir
from concourse.bass_utils import run_bass_kernel_spmd

# Problem shapes (hardcoded per instructions).
B = 2
S = 2048
D = 2048
H = 8
HD = 256
HALF = 128
EL = 512          # E columns per core (2 heads)
CH = 512          # sq chunk size
NCH = S // CH     # 4
DT = D // 128     # 16 k-tiles over D
ET = EL // 128    # 4 e-tiles
ST = S // 128     # 16 s-tiles
EPS = 1e-6
N_CORES = 8

BF16 = mybir.dt.bfloat16
F32 = mybir.dt.float32
I32 = mybir.dt.int32
NBF = ml_dtypes.bfloat16

SQRT_MAGIC = 0x1FBD1DF5     # sqrt(x) seed: (bits(x) >> 1) + MAGIC

_CACHE: dict = {}


def _build(reps: int = 1):
    nc = bacc.Bacc("TRN2", target_bir_lowering=False, debug=False,
                   num_devices=N_CORES)

    xT_d = nc.dram_tensor("xT", [D, S], BF16, kind="ExternalInput").ap()
    wq_d = nc.dram_tensor("wqT", [D, EL], BF16, kind="ExternalInput").ap()
    wk_d = nc.dram_tensor("wkT", [D, EL], BF16, kind="ExternalInput").ap()
    wv_d = nc.dram_tensor("wvT", [D, EL], BF16, kind="ExternalInput").ap()
    wo_d = nc.dram_tensor("woT", [EL, D], BF16, kind="ExternalInput").ap()
    rtq_d = nc.dram_tensor("rtq", [4, HALF, S], BF16, kind="ExternalInput").ap()
    rtk_d = nc.dram_tensor("rtk", [4, HALF, S], BF16, kind="ExternalInput").ap()
    msk_d = nc.dram_tensor("masks", [4, HALF, CH], BF16, kind="ExternalInput").ap()
    out_d = nc.dram_tensor("outT", [D, S], F32, kind="ExternalOutput").ap()

    with tile.TileContext(nc) as tc:
        for _ in range(reps):
            _emit(tc, nc, xT_d, wq_d, wk_d, wv_d, wo_d, rtq_d, rtk_d, msk_d,
                  out_d)
    nc.compile()
    return nc


def _emit(tc, nc, xT_d, wq_d, wk_d, wv_d, wo_d, rtq_d, rtk_d, msk_d, out_d):
    from contextlib import ExitStack
    ctx = ExitStack()
    with ctx:
        persist = ctx.enter_context(tc.tile_pool(name="persist", bufs=1))
        xs_p = ctx.enter_context(tc.tile_pool(name="xs", bufs=18))
        rt_p = ctx.enter_context(tc.tile_pool(name="rt", bufs=1))
        sq_p = ctx.enter_context(tc.tile_pool(name="sq", bufs=4))
        qr_p = ctx.enter_context(tc.tile_pool(name="qr", bufs=9))
        qs_p = ctx.enter_context(tc.tile_pool(name="qs", bufs=4))
        rtmp_p = ctx.enter_context(tc.tile_pool(name="rtmp", bufs=6))
        rb_p = ctx.enter_context(tc.tile_pool(name="rb", bufs=2))
        rd_p = ctx.enter_context(tc.tile_pool(name="rd", bufs=2))
        e_p = ctx.enter_context(tc.tile_pool(name="ep", bufs=8))
        o_p = ctx.enter_context(tc.tile_pool(name="op", bufs=3))
        nr_p = ctx.enter_context(tc.tile_pool(name="nrp", bufs=3))
        at_p = ctx.enter_context(tc.tile_pool(name="atp", bufs=2))

        ps_big = ctx.enter_context(tc.tile_pool(name="psb", bufs=6, space="PSUM"))
        ps_col = ctx.enter_context(tc.tile_pool(name="psc", bufs=2, space="PSUM"))

        # ---- persistent tiles ----
        wq_sb = persist.tile([128, DT, EL], BF16, tag="wq")
        wk_sb = persist.tile([128, DT, EL], BF16, tag="wk")
        wv_sb = persist.tile([128, DT, EL], BF16, tag="wv")
        wo_sb = persist.tile([128, ET, D], BF16, tag="wo")
        qT_sb = persist.tile([128, ET, S], BF16, tag="qT")
        kT_sb = persist.tile([128, ET, S], BF16, tag="kT")
        v_sb = persist.tile([128, ST, EL], BF16, tag="v")
        msk_sb = persist.tile([128, 4, CH], BF16, tag="msk")
        ones_sb = persist.tile([128, 1], BF16, tag="ones")
        rk_sb = persist.tile([128, 2, ST], F32, tag="rk")   # f_k per (head, sk)
        nc.vector.memset(ones_sb, 1.0)

        nc.sync.dma_start(out=wq_sb, in_=wq_d.rearrange("(t p) e -> p t e", p=128))

        def load_wk():
            nc.sync.dma_start(out=wk_sb, in_=wk_d.rearrange("(t p) e -> p t e", p=128))

        def load_wv():
            nc.sync.dma_start(out=wv_sb, in_=wv_d.rearrange("(t p) e -> p t e", p=128))

        def load_rest():
            nc.sync.dma_start(out=wo_sb, in_=wo_d.rearrange("(t p) d -> p t d", p=128))
            nc.sync.dma_start(out=msk_sb, in_=msk_d.rearrange("t p s -> p t s"))

        LN_EPS = float(HD * EPS)

        def nr_rsqrt(dst, src_ps, n, scale16):
            """dst[128, n] = (src_ps + 256*eps)^(-1/2) (*16), DVE only."""
            x = nr_p.tile([128, 8], F32, tag="nrx", name=f"nrx{nr_rsqrt.i}")[:, :n]
            nc.vector.tensor_scalar(out=x, in0=src_ps, scalar1=LN_EPS,
                                    scalar2=None, op0=mybir.AluOpType.add)
            s = nr_p.tile([128, 8], F32, tag="nry", name=f"nry{nr_rsqrt.i}")[:, :n]
            nc.vector.tensor_scalar(
                out=s.bitcast(I32), in0=x.bitcast(I32), scalar1=1,
                scalar2=None, op0=mybir.AluOpType.arith_shift_right)
            nc.vector.tensor_scalar(
                out=s.bitcast(I32), in0=s.bitcast(I32), scalar1=SQRT_MAGIC,
                scalar2=None, op0=mybir.AluOpType.add)
            for it in range(2):
                r = nr_p.tile([128, 8], F32, tag="nrt",
                              name=f"nrt{nr_rsqrt.i}_{it}")[:, :n]
                nc.vector.reciprocal(out=r, in_=s)
                nc.vector.tensor_mul(out=r, in0=r, in1=x)     # x / s
                nc.vector.tensor_add(out=s, in0=s, in1=r)
                nc.vector.tensor_scalar(out=s, in0=s, scalar1=0.5,
                                        scalar2=None, op0=mybir.AluOpType.mult)
            nr_rsqrt.i += 1
            nc.vector.reciprocal(out=dst, in_=s)
            if scale16:
                nc.vector.tensor_scalar(out=dst, in0=dst, scalar1=16.0,
                                        scalar2=None, op0=mybir.AluOpType.mult)

        nr_rsqrt.i = 0

        def col_to_row(src, n, eng=None):
            """[128, n] f32 cols -> [1, n*128] row on partition 0.

            row[0, 128*j + p] = src[p, j]"""
            row = nr_p.tile([1, 1024], F32, tag="row",
                            name=f"row{col_to_row.i}")
            col_to_row.i += 1
            eng = eng or nc.gpsimd
            for j in range(n):
                eng.dma_start(
                    out=row[:, j * 128:(j + 1) * 128], in_=src[:, j:j + 1])
            return row
        col_to_row.i = 0

        def _rope(dst_sb, et0, cs, x1, x2, tab):
            """dst[:, et0, cs]   = x1*tab[0] - x2*tab[3]   (cw1, sw2)
               dst[:, et0+1, cs] = x2*tab[2] + x1*tab[1]   (cw2, sw1)"""
            t1 = rtmp_p.tile([128, CH], BF16, tag="rtmp")
            t2 = rtmp_p.tile([128, CH], BF16, tag="rtmp")
            nc.vector.tensor_mul(out=t1, in0=x1, in1=tab[:, 0, :])
            nc.vector.tensor_mul(out=t2, in0=x2, in1=tab[:, 3, :])
            nc.vector.tensor_sub(out=dst_sb[:, et0, cs], in0=t1, in1=t2)
            t3 = rtmp_p.tile([128, CH], BF16, tag="rtmp")
            t4 = rtmp_p.tile([128, CH], BF16, tag="rtmp")
            nc.vector.tensor_mul(out=t3, in0=x2, in1=tab[:, 2, :])
            nc.vector.tensor_mul(out=t4, in0=x1, in1=tab[:, 1, :])
            nc.vector.tensor_add(out=dst_sb[:, et0 + 1, cs], in0=t3, in1=t4)

        def phase_a(c, first=False):
            """QKV projections + rmsnorm + rope for chunk c."""
            cs = slice(c * CH, (c + 1) * CH)
            xs = []
            for dt in range(DT):
                t = xs_p.tile([128, CH], BF16, tag="xs")
                nc.sync.dma_start(
                    out=t, in_=xT_d[dt * 128:(dt + 1) * 128, cs])
                xs.append(t)
            rtq_t = rt_p.tile([128, 4, CH], BF16, tag="rtq")
            nc.sync.dma_start(out=rtq_t,
                              in_=rtq_d[:, :, cs].rearrange("t p s -> p t s"))
            rtk_t = rt_p.tile([128, 4, CH], BF16, tag="rtk")
            nc.sync.dma_start(out=rtk_t,
                              in_=rtk_d[:, :, cs].rearrange("t p s -> p t s"))
            if first:
                load_wk()

            # ---------- q ----------
            rq_ps = ps_col.tile([128, 8], F32, tag="col")
            qr = []
            sqs = []
            for et in range(ET):
                q_ps = ps_big.tile([128, CH], F32, tag="big")
                for dt in range(DT):
                    nc.tensor.matmul(
                        q_ps, wq_sb[:, dt, et * 128:(et + 1) * 128], xs[dt],
                        start=(dt == 0), stop=(dt == DT - 1))
                sqt = sq_p.tile([128, CH], BF16, tag="sq")
                nc.scalar.activation(out=sqt, in_=q_ps,
                                     func=mybir.ActivationFunctionType.Square,
                                     bias=0.0, scale=1.0)
                sqs.append(sqt)
                t = qr_p.tile([128, CH], BF16, tag="qr")
                nc.scalar.copy(out=t, in_=q_ps)
                qr.append(t)
            def q_colmms():
                for et in range(ET):
                    hh = et // 2
                    for j in range(4):
                        nc.tensor.matmul(
                            rq_ps[:, 4 * hh + j: 4 * hh + j + 1],
                            sqs[et][:, j * 128:(j + 1) * 128], ones_sb,
                            start=(et == 0 and j == 0),
                            stop=(et == ET - 1 and j == 3))
            q_colmms()

            def q_tail():
                rq_sb = nr_p.tile([128, 8], F32, tag="rq")
                nr_rsqrt(rq_sb, rq_ps, 8, scale16=False)
                t_row = col_to_row(rq_sb, 8)
                for hh in range(2):
                    rbt = rb_p.tile([128, CH], F32, tag="rb")
                    nc.gpsimd.partition_broadcast(
                        rbt, t_row[0:1, hh * CH:(hh + 1) * CH])
                    q1 = qs_p.tile([128, CH], BF16, tag="qs")
                    nc.vector.tensor_mul(out=q1, in0=qr[2 * hh], in1=rbt)
                    q2 = qs_p.tile([128, CH], BF16, tag="qs")
                    nc.vector.tensor_mul(out=q2, in0=qr[2 * hh + 1], in1=rbt)
                    _rope(qT_sb, 2 * hh, cs, q1, q2, rtq_t)
            if first:
                load_wv()

            # ---------- k ----------
            rk_ps = ps_col.tile([128, 8], F32, tag="col")
            kr = []
            ksqs = []
            for et in range(ET):
                k_ps = ps_big.tile([128, CH], F32, tag="big")
                for dt in range(DT):
                    nc.tensor.matmul(
                        k_ps, wk_sb[:, dt, et * 128:(et + 1) * 128], xs[dt],
                        start=(dt == 0), stop=(dt == DT - 1))
                sqt = sq_p.tile([128, CH], BF16, tag="sq")
                nc.scalar.activation(out=sqt, in_=k_ps,
                                     func=mybir.ActivationFunctionType.Square,
                                     bias=0.0, scale=1.0)
                ksqs.append(sqt)
                t = qr_p.tile([128, CH], BF16, tag="qr")
                nc.scalar.copy(out=t, in_=k_ps)
                kr.append(t)
            def k_colmms():
                for et in range(ET):
                    hh = et // 2
                    for j in range(4):
                        nc.tensor.matmul(
                            rk_ps[:, 4 * hh + j: 4 * hh + j + 1],
                            ksqs[et][:, j * 128:(j + 1) * 128], ones_sb,
                            start=(et == 0 and j == 0),
                            stop=(et == ET - 1 and j == 3))
            k_colmms()

            def k_tail():
                nr_rsqrt(rk_sb[:, 0, 4 * c:4 * c + 4], rk_ps[:, 0:4], 4,
                         scale16=True)
                nr_rsqrt(rk_sb[:, 1, 4 * c:4 * c + 4], rk_ps[:, 4:8], 4,
                         scale16=True)
                for hh in range(2):
                    _rope(kT_sb, 2 * hh, cs, kr[2 * hh], kr[2 * hh + 1],
                          rtk_t)

            # ---------- v ----------
            for st in range(4):
                v_ps = ps_big.tile([128, EL], F32, tag="big")
                for dt in range(DT):
                    nc.tensor.matmul(
                        v_ps, xs[dt][:, st * 128:(st + 1) * 128],
                        wv_sb[:, dt, :],
                        start=(dt == 0), stop=(dt == DT - 1))
                nc.scalar.copy(out=v_sb[:, 4 * c + st, :], in_=v_ps)
            k_tail()
            if first:
                load_rest()

        def phase_b_pair(c, aT_t, filler=None):
            """Attention for chunk c, both local heads, skt-interleaved.

            filler: optional iterator yielding callables that emit a burst of
            independent PE work (used to keep the PE fed during the serial
            score->exp->mask->av chains)."""
            cs = slice(c * CH, (c + 1) * CH)
            n_sk = 4 * c + 4
            av_ps = {(hh, i): ps_big.tile([128, CH], F32, tag="big",
                                          name=f"av{c}_{hh}_{i}")
                     for hh in range(2) for i in range(2)}
            den_ps = {hh: ps_col.tile([128, 4], F32, tag="col",
                                      name=f"den{c}_{hh}")
                      for hh in range(2)}
            for skt in range(n_sk):
                first, last = (skt == 0), (skt == n_sk - 1)
                e_ts = {}
                for hh in range(2):
                    sc_ps = ps_big.tile([128, CH], F32, tag="big",
                                        name=f"sc{c}_{hh}_{skt}")
                    for half in range(2):
                        et = 2 * hh + half
                        nc.tensor.matmul(
                            sc_ps, kT_sb[:, et, skt * 128:(skt + 1) * 128],
                            qT_sb[:, et, cs],
                            start=(half == 0), stop=(half == 1))
                    e_t = e_p.tile([128, CH], BF16, tag="ep",
                                   name=f"et{c}_{hh}_{skt}")
                    nc.scalar.activation(out=e_t, in_=sc_ps,
                                         func=mybir.ActivationFunctionType.Exp,
                                         bias=0.0,
                                         scale=rk_sb[:, hh, skt:skt + 1])
                    rel = skt - 4 * c
                    if rel >= 0:
                        nc.vector.tensor_mul(out=e_t, in0=e_t,
                                             in1=msk_sb[:, rel, :])
                    e_ts[hh] = e_t
                if filler is not None and skt % 2 == 1:
                    for fn in next(filler, []) or []:
                        fn()
                for hh in range(2):
                    e_t = e_ts[hh]
                    for half in range(2):
                        nc.tensor.matmul(
                            av_ps[hh, half],
                            v_sb[:, skt, hh * 256 + half * 128:
                                 hh * 256 + (half + 1) * 128],
                            e_t, start=first, stop=last)
                    for j in range(4):
                        nc.tensor.matmul(
                            den_ps[hh][:, j:j + 1],
                            e_t[:, j * 128:(j + 1) * 128], ones_sb,
                            start=(first and j == 0), stop=(last and j == 3))
            for hh in range(2):
                dinv = nr_p.tile([128, 4], F32, tag="dinv",
                                 name=f"dinv{c}_{hh}")
                nc.vector.reciprocal(out=dinv, in_=den_ps[hh])
                t_row = col_to_row(dinv, 4)
                rd_t = rd_p.tile([128, CH], F32, tag="rd",
                                 name=f"rd{c}_{hh}")
                for j in range(4):
                    nc.gpsimd.partition_broadcast(
                        rd_t[:, j * 128:(j + 1) * 128],
                        t_row[0:1, j * 128:(j + 1) * 128])
                for half in range(2):
                    nc.vector.tensor_mul(out=aT_t[:, 2 * hh + half, :],
                                         in0=av_ps[hh, half], in1=rd_t)

        def c_burst(c, aT_t, dout):
            cs = slice(c * CH, (c + 1) * CH)
            o_ps = ps_big.tile([128, CH], F32, tag="big",
                               name=f"ops{c}_{dout}")
            for et in range(ET):
                nc.tensor.matmul(
                    o_ps, wo_sb[:, et, dout * 128:(dout + 1) * 128],
                    aT_t[:, et, :],
                    start=(et == 0), stop=(et == ET - 1))
            o_t = o_p.tile([128, CH], F32, tag="op", name=f"ot{c}_{dout}")
            if dout % 2 == 0:
                nc.vector.tensor_copy(out=o_t, in_=o_ps)
            else:
                nc.scalar.copy(out=o_t, in_=o_ps)
            nc.sync.dma_start(
                out=out_d[dout * 128:(dout + 1) * 128, cs], in_=o_t)

        def phase_c(c, aT_t):
            for dout in range(DT):
                c_burst(c, aT_t, dout)

        def c_filler(c, aT_t):
            for dout in range(DT):
                yield [lambda c=c, a=aT_t, d=dout: c_burst(c, a, d)]

        # ---- schedule ----
        aT = {}
        phase_a(0, first=True)
        phase_a(1)
        aT[0] = at_p.tile([128, ET, CH], BF16, tag="atp", name="aT0")
        phase_b_pair(0, aT[0])
        phase_a(2)
        aT[1] = at_p.tile([128, ET, CH], BF16, tag="atp", name="aT1")
        phase_b_pair(1, aT[1])
        phase_a(3)
        aT[2] = at_p.tile([128, ET, CH], BF16, tag="atp", name="aT2")
        phase_b_pair(2, aT[2], filler=c_filler(0, aT[0]))
        phase_c(0, aT[0])
        aT[3] = at_p.tile([128, ET, CH], BF16, tag="atp", name="aT3")
        phase_b_pair(3, aT[3], filler=c_filler(1, aT[1]))
        phase_c(1, aT[1])
        phase_c(2, aT[2])
        phase_c(3, aT[3])


# revision 29
# speedup vs baseline: 1.2525x; 1.0483x over previous
"""Multi-head causal attention (RoPE + QK-RMSNorm) on 8 TRN2 NeuronCores.

Sharding: data parallel on batch (2) x tensor parallel on heads (4 groups of
2 heads).  core = 4*b + g computes, for batch b, heads [2g, 2g+1]:
  q/k/v projections (E-sliced), qk-rmsnorm + rope, causal attention, and the
  Wo partial product over its E slice.  Host sums the 4 partials per batch.

Everything on device runs in the "transposed" orientation:
  qT/kT [e, s], v [s, e], scoresT [sk, sq], out_T [d_out, s]
so no on-chip data transposes are needed; softmax denominators and rms sums
are computed with ones-matmuls on the TensorEngine in *column* layout
[128 positions, few cols].  1/x uses the DVE divide unit; 1/sqrt(x) uses a
sqrt bit-hack seed + 2 Babylonian iterations + reciprocal, all on DVE, so
the ScalarEngine only ever needs the exp table set (no table thrashing).
Per-position column scales are turned into [1, n] rows with small gpsimd
DMAs and broadcast across partitions with gpsimd.partition_broadcast.

Scheduling (validated against the TimelineSim cost model): the attention
inner loop is software-pipelined depth-3 (scores k+3 issue before av/den k),
diagonal tiles compute only their live columns (cols < 128*rel are fully
masked and skipped in scores/exp/av/den), rms col-matmuls are hoisted
behind each projection section, the NR/rope tails are deferred past the v
section, Wo output bursts are injected as PE filler into the later
attention groups, and output stores alternate between the SP and ACT DMA
queues.  Estimated per-core exec ~308 us.
"""

import math

import numpy as np
import ml_dtypes

import concourse.bass as bass
import concourse.tile as tile
from concourse import bacc, mybir
from concourse.bass_utils import run_bass_kernel_spmd

# Problem shapes (hardcoded per instructions).
B = 2
S = 2048
D = 2048
H = 8
HD = 256
HALF = 128
EL = 512          # E columns per core (2 heads)
CH = 512          # sq chunk size
NCH = S // CH     # 4
DT = D // 128     # 16 k-tiles over D
ET = EL // 128    # 4 e-tiles
ST = S // 128     # 16 s-tiles
EPS = 1e-6
N_CORES = 8

BF16 = mybir.dt.bfloat16
F32 = mybir.dt.float32
I32 = mybir.dt.int32
NBF = ml_dtypes.bfloat16

SQRT_MAGIC = 0x1FBD1DF5     # sqrt(x) seed: (bits(x) >> 1) + MAGIC

_CACHE: dict = {}


def _build(reps: int = 1):
    nc = bacc.Bacc("TRN2", target_bir_lowering=False, debug=False,
                   num_devices=N_CORES)

    xT_d = nc.dram_tensor("xT", [D, S], BF16, kind="ExternalInput").ap()
    wq_d = nc.dram_tensor("wqT", [D, EL], BF16, kind="ExternalInput").ap()
    wk_d = nc.dram_tensor("wkT", [D, EL], BF16, kind="ExternalInput").ap()
    wv_d = nc.dram_tensor("wvT", [D, EL], BF16, kind="ExternalInput").ap()
    wo_d = nc.dram_tensor("woT", [EL, D], BF16, kind="ExternalInput").ap()
    rtq_d = nc.dram_tensor("rtq", [4, HALF, S], BF16, kind="ExternalInput").ap()
    rtk_d = nc.dram_tensor("rtk", [4, HALF, S], BF16, kind="ExternalInput").ap()
    msk_d = nc.dram_tensor("masks", [4, HALF, CH], BF16, kind="ExternalInput").ap()
    out_d = nc.dram_tensor("outT", [D, S], F32, kind="ExternalOutput").ap()

    with tile.TileContext(nc) as tc:
        for _ in range(reps):
            _emit(tc, nc, xT_d, wq_d, wk_d, wv_d, wo_d, rtq_d, rtk_d, msk_d,
                  out_d)
    nc.compile()
    return nc


def _emit(tc, nc, xT_d, wq_d, wk_d, wv_d, wo_d, rtq_d, rtk_d, msk_d, out_d):
    from contextlib import ExitStack
    ctx = ExitStack()
    with ctx:
        persist = ctx.enter_context(tc.tile_pool(name="persist", bufs=1))
        xs_p = ctx.enter_context(tc.tile_pool(name="xs", bufs=18))
        rt_p = ctx.enter_context(tc.tile_pool(name="rt", bufs=1))
        sq_p = ctx.enter_context(tc.tile_pool(name="sq", bufs=4))
        qr_p = ctx.enter_context(tc.tile_pool(name="qr", bufs=9))
        qs_p = ctx.enter_context(tc.tile_pool(name="qs", bufs=4))
        rtmp_p = ctx.enter_context(tc.tile_pool(name="rtmp", bufs=5))
        rb_p = ctx.enter_context(tc.tile_pool(name="rb", bufs=2))
        rd_p = ctx.enter_context(tc.tile_pool(name="rd", bufs=2))
        e_p = ctx.enter_context(tc.tile_pool(name="ep", bufs=7))
        o_p = ctx.enter_context(tc.tile_pool(name="op", bufs=4))
        nr_p = ctx.enter_context(tc.tile_pool(name="nrp", bufs=3))
        at_p = ctx.enter_context(tc.tile_pool(name="atp", bufs=2))

        ps_big = ctx.enter_context(tc.tile_pool(name="psb", bufs=6, space="PSUM"))
        ps_col = ctx.enter_context(tc.tile_pool(name="psc", bufs=2, space="PSUM"))

        # ---- persistent tiles ----
        wq_sb = persist.tile([128, DT, EL], BF16, tag="wq")
        wk_sb = persist.tile([128, DT, EL], BF16, tag="wk")
        wv_sb = persist.tile([128, DT, EL], BF16, tag="wv")
        wo_sb = persist.tile([128, ET, D], BF16, tag="wo")
        qT_sb = persist.tile([128, ET, S], BF16, tag="qT")
        kT_sb = persist.tile([128, ET, S], BF16, tag="kT")
        v_sb = persist.tile([128, ST, EL], BF16, tag="v")
        msk_sb = persist.tile([128, 4, CH], BF16, tag="msk")
        ones_sb = persist.tile([128, 1], BF16, tag="ones")
        rk_sb = persist.tile([128, 2, ST], F32, tag="rk")   # f_k per (head, sk)
        nc.vector.memset(ones_sb, 1.0)

        nc.sync.dma_start(out=wq_sb, in_=wq_d.rearrange("(t p) e -> p t e", p=128))

        def load_wk():
            nc.sync.dma_start(out=wk_sb, in_=wk_d.rearrange("(t p) e -> p t e", p=128))

        def load_wv():
            nc.sync.dma_start(out=wv_sb, in_=wv_d.rearrange("(t p) e -> p t e", p=128))

        def load_rest():
            nc.sync.dma_start(out=wo_sb, in_=wo_d.rearrange("(t p) d -> p t d", p=128))
            nc.sync.dma_start(out=msk_sb, in_=msk_d.rearrange("t p s -> p t s"))

        LN_EPS = float(HD * EPS)

        def nr_rsqrt(dst, src_ps, n, scale16):
            """dst[128, n] = (src_ps + 256*eps)^(-1/2) (*16), DVE only."""
            x = nr_p.tile([128, 8], F32, tag="nrx", name=f"nrx{nr_rsqrt.i}")[:, :n]
            nc.vector.tensor_scalar(out=x, in0=src_ps, scalar1=LN_EPS,
                                    scalar2=None, op0=mybir.AluOpType.add)
            s = nr_p.tile([128, 8], F32, tag="nry", name=f"nry{nr_rsqrt.i}")[:, :n]
            nc.vector.tensor_scalar(
                out=s.bitcast(I32), in0=x.bitcast(I32), scalar1=1,
                scalar2=None, op0=mybir.AluOpType.arith_shift_right)
            nc.vector.tensor_scalar(
                out=s.bitcast(I32), in0=s.bitcast(I32), scalar1=SQRT_MAGIC,
                scalar2=None, op0=mybir.AluOpType.add)
            for it in range(2):
                r = nr_p.tile([128, 8], F32, tag="nrt",
                              name=f"nrt{nr_rsqrt.i}_{it}")[:, :n]
                nc.vector.reciprocal(out=r, in_=s)
                nc.vector.tensor_mul(out=r, in0=r, in1=x)     # x / s
                nc.vector.tensor_add(out=s, in0=s, in1=r)
                nc.vector.tensor_scalar(out=s, in0=s, scalar1=0.5,
                                        scalar2=None, op0=mybir.AluOpType.mult)
            nr_rsqrt.i += 1
            nc.vector.reciprocal(out=dst, in_=s)
            if scale16:
                nc.vector.tensor_scalar(out=dst, in0=dst, scalar1=16.0,
                                        scalar2=None, op0=mybir.AluOpType.mult)

        nr_rsqrt.i = 0

        def col_to_row(src, n, eng=None):
            """[128, n] f32 cols -> [1, n*128] row on partition 0.

            row[0, 128*j + p] = src[p, j]"""
            row = nr_p.tile([1, 1024], F32, tag="row",
                            name=f"row{col_to_row.i}")
            col_to_row.i += 1
            eng = eng or nc.gpsimd
            for j in range(n):
                eng.dma_start(
                    out=row[:, j * 128:(j + 1) * 128], in_=src[:, j:j + 1])
            return row
        col_to_row.i = 0

        def _rope(dst_sb, et0, cs, x1, x2, tab):
            """dst[:, et0, cs]   = x1*tab[0] - x2*tab[3]   (cw1, sw2)
               dst[:, et0+1, cs] = x2*tab[2] + x1*tab[1]   (cw2, sw1)"""
            t1 = rtmp_p.tile([128, CH], BF16, tag="rtmp")
            t2 = rtmp_p.tile([128, CH], BF16, tag="rtmp")
            nc.vector.tensor_mul(out=t1, in0=x1, in1=tab[:, 0, :])
            nc.vector.tensor_mul(out=t2, in0=x2, in1=tab[:, 3, :])
            nc.vector.tensor_sub(out=dst_sb[:, et0, cs], in0=t1, in1=t2)
            t3 = rtmp_p.tile([128, CH], BF16, tag="rtmp")
            t4 = rtmp_p.tile([128, CH], BF16, tag="rtmp")
            nc.vector.tensor_mul(out=t3, in0=x2, in1=tab[:, 2, :])
            nc.vector.tensor_mul(out=t4, in0=x1, in1=tab[:, 1, :])
            nc.vector.tensor_add(out=dst_sb[:, et0 + 1, cs], in0=t3, in1=t4)

        def phase_a(c, first=False):
            """QKV projections + rmsnorm + rope for chunk c."""
            cs = slice(c * CH, (c + 1) * CH)
            xs = []
            for dt in range(DT):
                t = xs_p.tile([128, CH], BF16, tag="xs")
                nc.sync.dma_start(
                    out=t, in_=xT_d[dt * 128:(dt + 1) * 128, cs])
                xs.append(t)
            rtq_t = rt_p.tile([128, 4, CH], BF16, tag="rtq")
            nc.sync.dma_start(out=rtq_t,
                              in_=rtq_d[:, :, cs].rearrange("t p s -> p t s"))
            rtk_t = rt_p.tile([128, 4, CH], BF16, tag="rtk")
            nc.sync.dma_start(out=rtk_t,
                              in_=rtk_d[:, :, cs].rearrange("t p s -> p t s"))
            if first:
                load_wk()

            # ---------- q ----------
            rq_ps = ps_col.tile([128, 8], F32, tag="col")
            qr = []
            sqs = []
            for et in range(ET):
                q_ps = ps_big.tile([128, CH], F32, tag="big")
                for dt in range(DT):
                    nc.tensor.matmul(
                        q_ps, wq_sb[:, dt, et * 128:(et + 1) * 128], xs[dt],
                        start=(dt == 0), stop=(dt == DT - 1))
                sqt = sq_p.tile([128, CH], BF16, tag="sq")
                nc.scalar.activation(out=sqt, in_=q_ps,
                                     func=mybir.ActivationFunctionType.Square,
                                     bias=0.0, scale=1.0)
                sqs.append(sqt)
                t = qr_p.tile([128, CH], BF16, tag="qr")
                nc.scalar.copy(out=t, in_=q_ps)
                qr.append(t)
            def q_colmms():
                for et in range(ET):
                    hh = et // 2
                    for j in range(4):
                        nc.tensor.matmul(
                            rq_ps[:, 4 * hh + j: 4 * hh + j + 1],
                            sqs[et][:, j * 128:(j + 1) * 128], ones_sb,
                            start=(et == 0 and j == 0),
                            stop=(et == ET - 1 and j == 3))
            q_colmms()

            def q_tail():
                rq_sb = nr_p.tile([128, 8], F32, tag="rq")
                nr_rsqrt(rq_sb, rq_ps, 8, scale16=False)
                t_row = col_to_row(rq_sb, 8)
                for hh in range(2):
                    rbt = rb_p.tile([128, CH], F32, tag="rb")
                    nc.gpsimd.partition_broadcast(
                        rbt, t_row[0:1, hh * CH:(hh + 1) * CH])
                    q1 = qs_p.tile([128, CH], BF16, tag="qs")
                    nc.vector.tensor_mul(out=q1, in0=qr[2 * hh], in1=rbt)
                    q2 = qs_p.tile([128, CH], BF16, tag="qs")
                    nc.vector.tensor_mul(out=q2, in0=qr[2 * hh + 1], in1=rbt)
                    _rope(qT_sb, 2 * hh, cs, q1, q2, rtq_t)
            if first:
                load_wv()

            # ---------- k ----------
            rk_ps = ps_col.tile([128, 8], F32, tag="col")
            kr = []
            ksqs = []
            for et in range(ET):
                k_ps = ps_big.tile([128, CH], F32, tag="big")
                for dt in range(DT):
                    nc.tensor.matmul(
                        k_ps, wk_sb[:, dt, et * 128:(et + 1) * 128], xs[dt],
                        start=(dt == 0), stop=(dt == DT - 1))
                sqt = sq_p.tile([128, CH], BF16, tag="sq")
                nc.scalar.activation(out=sqt, in_=k_ps,
                                     func=mybir.ActivationFunctionType.Square,
                                     bias=0.0, scale=1.0)
                ksqs.append(sqt)
                t = qr_p.tile([128, CH], BF16, tag="qr")
                nc.scalar.copy(out=t, in_=k_ps)
                kr.append(t)
            def k_colmms():
                for et in range(ET):
                    hh = et // 2
                    for j in range(4):
                        nc.tensor.matmul(
                            rk_ps[:, 4 * hh + j: 4 * hh + j + 1],
                            ksqs[et][:, j * 128:(j + 1) * 128], ones_sb,
                            start=(et == 0 and j == 0),
                            stop=(et == ET - 1 and j == 3))
            k_colmms()

            def k_tail():
                nr_rsqrt(rk_sb[:, 0, 4 * c:4 * c + 4], rk_ps[:, 0:4], 4,
                         scale16=True)
                nr_rsqrt(rk_sb[:, 1, 4 * c:4 * c + 4], rk_ps[:, 4:8], 4,
                         scale16=True)
                for hh in range(2):
                    _rope(kT_sb, 2 * hh, cs, kr[2 * hh], kr[2 * hh + 1],
                          rtk_t)

            # ---------- v ----------
            for st in range(4):
                v_ps = ps_big.tile([128, EL], F32, tag="big")
                for dt in range(DT):
                    nc.tensor.matmul(
                        v_ps, xs[dt][:, st * 128:(st + 1) * 128],
                        wv_sb[:, dt, :],
                        start=(dt == 0), stop=(dt == DT - 1))
                nc.scalar.copy(out=v_sb[:, 4 * c + st, :], in_=v_ps)
            k_tail()
            if first:
                load_rest()

        def phase_b_pair(c, aT_t, filler=None):
            """Attention for chunk c, both local heads, skt-interleaved.

            filler: optional iterator yielding callables that emit a burst of
            independent PE work (used to keep the PE fed during the serial
            score->exp->mask->av chains)."""
            cs = slice(c * CH, (c + 1) * CH)
            n_sk = 4 * c + 4
            av_ps = {(hh, i): ps_big.tile([128, CH], F32, tag="big",
                                          name=f"av{c}_{hh}_{i}")
                     for hh in range(2) for i in range(2)}
            den_ps = {hh: ps_col.tile([128, 4], F32, tag="col",
                                      name=f"den{c}_{hh}")
                      for hh in range(2)}
            for skt in range(n_sk):
                first, last = (skt == 0), (skt == n_sk - 1)
                e_ts = {}
                for hh in range(2):
                    sc_ps = ps_big.tile([128, CH], F32, tag="big",
                                        name=f"sc{c}_{hh}_{skt}")
                    for half in range(2):
                        et = 2 * hh + half
                        nc.tensor.matmul(
                            sc_ps, kT_sb[:, et, skt * 128:(skt + 1) * 128],
                            qT_sb[:, et, cs],
                            start=(half == 0), stop=(half == 1))
                    e_t = e_p.tile([128, CH], BF16, tag="ep",
                                   name=f"et{c}_{hh}_{skt}")
                    nc.scalar.activation(out=e_t, in_=sc_ps,
                                         func=mybir.ActivationFunctionType.Exp,
                                         bias=0.0,
                                         scale=rk_sb[:, hh, skt:skt + 1])
                    rel = skt - 4 * c
                    if rel >= 0:
                        nc.vector.tensor_mul(out=e_t, in0=e_t,
                                             in1=msk_sb[:, rel, :])
                    e_ts[hh] = e_t
                if filler is not None and skt % 2 == 1:
                    for fn in next(filler, []) or []:
                        fn()
                for hh in range(2):
                    e_t = e_ts[hh]
                    for half in range(2):
                        nc.tensor.matmul(
                            av_ps[hh, half],
                            v_sb[:, skt, hh * 256 + half * 128:
                                 hh * 256 + (half + 1) * 128],
                            e_t, start=first, stop=last)
                    for j in range(4):
                        nc.tensor.matmul(
                            den_ps[hh][:, j:j + 1],
                            e_t[:, j * 128:(j + 1) * 128], ones_sb,
                            start=(first and j == 0), stop=(last and j == 3))
            for hh in range(2):
                dinv = nr_p.tile([128, 4], F32, tag="dinv",
                                 name=f"dinv{c}_{hh}")
                nc.vector.reciprocal(out=dinv, in_=den_ps[hh])
                t_row = col_to_row(dinv, 4)
                rd_t = rd_p.tile([128, CH], F32, tag="rd",
                                 name=f"rd{c}_{hh}")
                for j in range(4):
                    nc.gpsimd.partition_broadcast(
                        rd_t[:, j * 128:(j + 1) * 128],
                        t_row[0:1, j * 128:(j + 1) * 128])
                for half in range(2):
                    nc.vector.tensor_mul(out=aT_t[:, 2 * hh + half, :],
                                         in0=av_ps[hh, half], in1=rd_t)

        def c_burst(c, aT_t, dout):
            cs = slice(c * CH, (c + 1) * CH)
            o_ps = ps_big.tile([128, CH], F32, tag="big",
                               name=f"ops{c}_{dout}")
            for et in range(ET):
                nc.tensor.matmul(
                    o_ps, wo_sb[:, et, dout * 128:(dout + 1) * 128],
                    aT_t[:, et, :],
                    start=(et == 0), stop=(et == ET - 1))
            o_t = o_p.tile([128, CH], F32, tag="op", name=f"ot{c}_{dout}")
            if dout % 2 == 0:
                nc.vector.tensor_copy(out=o_t, in_=o_ps)
            else:
                nc.scalar.copy(out=o_t, in_=o_ps)
            eng = nc.sync if dout % 2 == 0 else nc.scalar
            eng.dma_start(
                out=out_d[dout * 128:(dout + 1) * 128, cs], in_=o_t)

        def phase_c(c, aT_t):
            for dout in range(DT):
                c_burst(c, aT_t, dout)

        def c_filler(c, aT_t):
            for dout in range(DT):
                yield [lambda c=c, a=aT_t, d=dout: c_burst(c, a, d)]

        # ---- schedule ----
        aT = {}
        phase_a(0, first=True)
        phase_a(1)
        aT[0] = at_p.tile([128, ET, CH], BF16, tag="atp", name="aT0")
        phase_b_pair(0, aT[0])
        phase_a(2)
        aT[1] = at_p.tile([128, ET, CH], BF16, tag="atp", name="aT1")
        phase_b_pair(1, aT[1])
        phase_a(3)
        aT[2] = at_p.tile([128, ET, CH], BF16, tag="atp", name="aT2")
        phase_b_pair(2, aT[2], filler=c_filler(0, aT[0]))
        phase_c(0, aT[0])
        aT[3] = at_p.tile([128, ET, CH], BF16, tag="atp", name="aT3")
        phase_b_pair(3, aT[3], filler=c_filler(1, aT[1]))
        phase_c(1, aT[1])
        phase_c(2, aT[2])
        phase_c(3, aT[3])
